# revision 14
# baseline (speedup 1.0000x reference)
"""Trainium2 Bass kernel for the 81-step LSTM decoder + masked softmax.

Math (per batch row b):
    z_t = x_t @ W_x + h_{t-1} @ W_h + b          (gates i, f, g, o; 100 each)
    i,f,o = sigmoid;  g = identity
    c_t = f*c_{t-1} + i*g;  h_t = o*c_t
    out_t = softmax(where(mask_t, h_t, -inf))

Strategy: data-parallel over batch (4096 -> 8 cores x 512); no collectives.

Device layout is BATCH-major: z is computed as [128b, 400gate] per batch
tile (4 tiles of 128), with x^T chunks as the stationary operand and the
weights as the 400-wide moving operand -- 400 < 512 moving columns puts the
xW GEMM at the PE's 100%-utilization floor (vs streaming the 512-wide batch).

Gates use the identity sigmoid(z) = 0.5*(tanh(z/2)+1).  tanh and exp live in
the SAME activation table ("exp_and_others"), so the kernel performs zero
ACT table reloads.  The 0.5/+1 affines are folded away:
  - device state is h' = 2h; host pre-scales W_h by 0.5 (and h0 by 2)
  - g columns of W_x/W_h/b are pre-scaled by 0.5 so (y_i+1)*g' = i*g
  - softmax uses exp(0.5*h') = exp(h)
Per tile: y = tanh(0.5*z[ifo]); v=(y_i+1)*g'; u=(y_f+1)*c; c=0.5u+v;
h'=(y_o+1)*c.  Masked softmax is batch-major: e = exp(0.5h'), em = e*mask
with a fused row-sum (accum_out), out = em * (1/s) on the Pool engine.
The only transposes are 4 small PE transposes h'[128,100] -> hist[100,128]
feeding the next step's recurrent matmul.
"""

import sys

if "/opt/trn_rl_repo" not in sys.path:
    sys.path.insert(0, "/opt/trn_rl_repo")

import numpy as np

P = 81       # places / timesteps
H = 100      # LSTM units
E = 512      # encoder feature width
B = 4096     # total batch
NCORES = 8
BS = B // NCORES          # 512 batch rows per core
NB = BS // 128            # 4 batch tiles of 128
NE = E // 128             # 4 feature chunks of 128
NHIST = 3                 # recurrent-state ring depth

_PROGRAM = None


def _build_program():
    import concourse.bacc as bacc
    import concourse.mybir as mybir
    from concourse.tile import TileContext
    from contextlib import ExitStack

    f32 = mybir.dt.float32
    f32r = mybir.dt.float32r
    bf16 = mybir.dt.bfloat16
    TANH = mybir.ActivationFunctionType.Tanh
    EXP = mybir.ActivationFunctionType.Exp
    ADD = mybir.AluOpType.add
    MULT = mybir.AluOpType.mult
    X = mybir.AxisListType.X

    nc = bacc.Bacc(None, target_bir_lowering=False)

    # packed f32r consts along free dim: ident [0:128], whb [128:528],
    # mask [528:528+81*100]
    C_ID = 0
    C_WHB = 128
    C_MB = C_WHB + 400
    C_TOT = C_MB + P * H
    xT_d = nc.dram_tensor("xT", [P, E, BS], bf16, kind="ExternalInput")
    wxb_d = nc.dram_tensor("wxb", [128, NE * 400], bf16, kind="ExternalInput")
    consts_d = nc.dram_tensor("consts", [128, C_TOT], f32r, kind="ExternalInput")
    h0T_d = nc.dram_tensor("h0T", [H + 1, BS], f32r, kind="ExternalInput")
    out_d = nc.dram_tensor("out", [BS, P, H], f32, kind="ExternalOutput")

    with ExitStack() as ctx:
        tc = ctx.enter_context(TileContext(nc))
        consts = ctx.enter_context(tc.tile_pool(name="consts", bufs=1))
        xpool = ctx.enter_context(tc.tile_pool(name="xpool", bufs=14))
        ypool = ctx.enter_context(tc.tile_pool(name="ypool", bufs=3))
        gpool = ctx.enter_context(tc.tile_pool(name="gpool", bufs=3))
        hpool = ctx.enter_context(tc.tile_pool(name="hpool", bufs=4))
        opool = ctx.enter_context(tc.tile_pool(name="opool", bufs=3))
        zpool = ctx.enter_context(tc.tile_pool(name="zpool", bufs=6, space="PSUM"))
        htpool = ctx.enter_context(tc.tile_pool(name="htpool", bufs=2, space="PSUM"))

        # ---- one-time loads ----
        csb = consts.tile([128, C_TOT], f32r)
        # identity + whb first (needed by step 0); mask later (needed by tails)
        nc.sync.dma_start(out=csb[:, 0:C_MB], in_=consts_d[:, 0:C_MB])
        wxb = consts.tile([128, NE, 400], bf16)
        nc.sync.dma_start(
            out=wxb, in_=wxb_d.rearrange("p (c g) -> p c g", c=NE)
        )
        nc.sync.dma_start(out=csb[:, C_MB:C_TOT], in_=consts_d[:, C_MB:C_TOT])
        idn = csb[:, C_ID : C_ID + 128]
        whb = csb[0 : H + 1, C_WHB : C_WHB + 400]
        maskb = csb.bitcast(f32)[:, C_MB:C_TOT].rearrange("p (t h) -> p t h", t=P)

        # recurrent-state ring h'^T [101, 512]; row H = 1.0 (bias rider)
        hist = [consts.tile([H + 1, BS], f32r, name=f"hist{j}") for j in range(NHIST)]
        for j in range(NHIST - 1):
            nc.sync.dma_start(out=hist[j][H : H + 1, :], in_=h0T_d[H : H + 1, :])
        nc.sync.dma_start(out=hist[NHIST - 1], in_=h0T_d[:, :])
        cT = consts.tile([128, NB, H], f32)      # persistent cell state
        nc.vector.memset(cT, 0.0)

        hprev = [None]  # h' tile of the previous step

        def emit_tail(t, hp):
            """softmax tail for step t: e = exp(h'/2) (ACT, fused over
            tiles); em = e*mask (Pool, plain tensor_tensor); s = row-sum
            (DVE reduce); ot = em * (1/s) (Pool, broadcast multiplicand).
            Emitted after step t+1's gate ops: every engine sees the
            recurrence-critical work first in its queue."""
            e = gpool.tile([128, NB, H], f32, name=f"e_{t}", tag="e")
            nc.scalar.activation(e, hp.bitcast(f32), EXP, scale=0.5)
            em = gpool.tile([128, NB, H], f32, name=f"em_{t}", tag="em")
            for k in range(NB):
                nc.gpsimd.tensor_mul(em[:, k, :], e[:, k, :], maskb[:, t, :])
            s = opool.tile([128, NB], f32, name=f"s_{t}", tag="s")
            nc.vector.tensor_reduce(s, em, axis=X, op=ADD)
            r = opool.tile([128, NB], f32, name=f"r_{t}", tag="r")
            nc.vector.reciprocal(r, s)
            ot = gpool.tile([128, NB, H], f32, name=f"ot_{t}", tag="ot")
            for k in range(NB):
                nc.gpsimd.tensor_mul(
                    ot[:, k, :], em[:, k, :], r[:, k : k + 1].broadcast_to([128, H])
                )
            nc.sync.dma_start(
                out=out_d[:, t, :].rearrange("(k p) h -> p k h", p=128), in_=ot
            )

        for t in range(P):
            # ---- stream x_t^T in, feature chunks on partitions (one DMA) ----
            xtile = xpool.tile([128, NE, BS], bf16, name=f"x_{t}", tag="x")
            nc.sync.dma_start(
                out=xtile, in_=xT_d[t].rearrange("(c p) b -> p c b", p=128)
            )

            # ---- PE stream: xW tiles 0,1 | transposes(t-1) | xW tile 2,
            # hW 0-2 | xW tile 3 | hW 3.  Per-tile accumulation groups keep
            # xW (start) before hW (stop); hW_k depends on hist(t-1). ----
            z = [
                zpool.tile([128, 400], f32, name=f"z_{t}_{k}", tag="z")
                for k in range(NB)
            ]

            def xw(k):
                for ec in range(NE):
                    nc.tensor.matmul(
                        z[k],
                        xtile[:, ec, 128 * k : 128 * (k + 1)],
                        wxb[:, ec, :],
                        start=(ec == 0),
                        stop=False,
                    )

            def hw(k):
                nc.tensor.matmul(
                    z[k],
                    hist[(t - 1) % NHIST][:, 128 * k : 128 * (k + 1)],
                    whb,
                    start=False,
                    stop=True,
                )

            xw(0)
            xw(1)
            xw(2)
            if t >= 1 and hprev[0] is not None:
                # The 4 batch tiles are independent recurrences: transpose
                # and copy PER TILE (subtile deps) so each tile's chain
                # transp_k -> copy_k -> hW_k(t) closes without waiting on
                # the other tiles.
                htp = htpool.tile([H, BS], f32r, name=f"htp_{t}", tag="htp")
                hdst = hist[(t - 1) % NHIST]
                for p in range(NB // 2):
                    for k in (2 * p, 2 * p + 1):
                        sl = slice(128 * k, 128 * (k + 1))
                        nc.tensor.transpose(htp[:, sl], hprev[0][:, k, :], idn)
                    # PSUM -> SBUF so hW_k can use it as stationary
                    # (ACT: GpSimd cannot access PSUM, DVE is busiest)
                    sp = slice(256 * p, 256 * (p + 1))
                    nc.scalar.copy(hdst[0:H, sp], htp[:, sp])
            xw(3)
            hw(0)
            hw(1)
            hw(2)
            hw(3)

            # ---- gates: y = tanh(0.5*z[:, ifo]) ----
            y = ypool.tile([128, NB, 300], f32, name=f"y_{t}", tag="y")
            for k in range(NB):
                nc.scalar.activation(y[:, k, :], z[k][:, 0:300], TANH, scale=0.5)

            u = gpool.tile([128, NB, H], f32, name=f"u_{t}", tag="u")
            v = gpool.tile([128, NB, H], f32, name=f"v_{t}", tag="v")
            hq = hpool.tile([128, NB, H], f32r, name=f"h_{t}", tag="h")
            # all on DVE (fused scalar_tensor_tensor is DVE-only).  v is
            # per-tile (reads a per-tile PSUM bank); u/c/h' fuse tile PAIRS
            # to cut DVE instruction count while following the tanh stagger.
            for p in range(NB // 2):
                for k in (2 * p, 2 * p + 1):
                    # v = (y_i + 1) * g'  (g' = 0.5*g via host weight scale)
                    nc.vector.scalar_tensor_tensor(
                        v[:, k, :], y[:, k, 0:100], 1.0, z[k][:, 300:400],
                        ADD, MULT,
                    )
                pr = slice(2 * p, 2 * p + 2)
                # u = (y_f + 1) * c_{t-1}
                nc.vector.scalar_tensor_tensor(
                    u[:, pr, :], y[:, pr, 100:200], 1.0, cT[:, pr, :], ADD, MULT
                )
                # c = 0.5*u + v
                nc.vector.scalar_tensor_tensor(
                    cT[:, pr, :], u[:, pr, :], 0.5, v[:, pr, :], MULT, ADD
                )
                # h' = (y_o + 1) * c   (= 2*h)
                nc.vector.scalar_tensor_tensor(
                    hq[:, pr, :], y[:, pr, 200:300], 1.0, cT[:, pr, :], ADD, MULT
                )

            # ---- softmax tail of the previous step ----
            if t >= 1:
                emit_tail(t - 1, hprev[0])

            # h'^T for the next step's recurrent matmul is produced at the
            # top of iteration t+1 (transposes interleaved into the PE
            # stream); stash h' into the ring slot t % NHIST there.
            hprev[0] = hq

        emit_tail(P - 1, hprev[0])

    nc.compile()
    return nc


def _get_program():
    global _PROGRAM
    if _PROGRAM is None:
        _PROGRAM = _build_program()
    return _PROGRAM


def _prep_in_maps(h_enc, h0, W_x, W_h, b, mask):
    h_enc = np.asarray(h_enc, dtype=np.float32)
    h0 = np.asarray(h0, dtype=np.float32)
    W_x = np.asarray(W_x, dtype=np.float32)
    W_h = np.asarray(W_h, dtype=np.float32)
    b = np.asarray(b, dtype=np.float32)
    mask = np.asarray(mask)

    # gate reorder i,f,g,o -> i,f,o,g and fold the tanh/2h rescales into
    # the weights (see module docstring)
    perm = np.concatenate(
        [np.arange(0, 200), np.arange(300, 400), np.arange(200, 300)]
    )
    gscale = np.ones((400,), np.float32)
    gscale[300:400] = 0.5                      # g' = 0.5*g
    Wx_dev = W_x[:, perm] * gscale             # [512, 400]
    Wh_dev = 0.5 * W_h[:, perm] * gscale       # h enters as h' = 2h
    b_dev = b[perm] * gscale

    # moving-operand layout for the xW matmuls: [128, NE, 400]
    wx_sb = Wx_dev.reshape(NE, 128, 400).transpose(1, 0, 2).reshape(128, NE * 400)

    C_TOT = 128 + 400 + P * H
    consts = np.zeros((128, C_TOT), np.float32)
    consts[:, 0:128] = np.eye(128, dtype=np.float32)
    consts[0:H, 128:528] = Wh_dev
    consts[H, 128:528] = b_dev
    maskf = np.where(mask, 1.0, 0.0).astype(np.float32).reshape(1, P * H)
    consts[:, 528:] = maskf  # broadcast to all 128 partitions

    import ml_dtypes

    bf16 = ml_dtypes.bfloat16
    wxb = np.ascontiguousarray(wx_sb).astype(bf16)
    in_maps = []
    xTf = np.empty((P, E, BS), np.float32)
    for c in range(NCORES):
        shard = h_enc[c * BS : (c + 1) * BS]  # [BS, P, E]
        for t in range(P):
            xTf[t] = shard[:, t, :].T
        xT = xTf.astype(bf16)
        h0T = np.ascontiguousarray(
            np.concatenate(
                [2.0 * h0[c * BS : (c + 1) * BS].T, np.ones((1, BS), np.float32)],
                axis=0,
            )
        )
        in_maps.append({"xT": xT, "wxb": wxb, "consts": consts, "h0T": h0T})
    return in_maps


def run(inputs: dict, trace: bool = False):
    """Run on 8 cores; returns (full_output, exec_time_ns_or_None)."""
    from concourse.bass_utils import run_bass_kernel_spmd

    nc = _get_program()
    in_maps = _prep_in_maps(**inputs)
    res = run_bass_kernel_spmd(
        nc, in_maps, core_ids=list(range(NCORES)), trace=trace
    )
    out = np.concatenate([r["out"] for r in res.results], axis=0)
    return out, res.exec_time_ns


def kernel(**inputs) -> np.ndarray:
    out, _ = run(inputs, trace=False)
    return out
